# revision 38
# baseline (speedup 1.0000x reference)
"""Trainium2 Bass kernel for nn_AutoShot (histogram binning + windowed similarity + FC).

Sharding: data-parallel over B*T = 400 frames -> 8 cores x 50 frames.
Phase A (heavy): per-core color histograms [50, 512] via
  bin = (R>>5)<<6 | (G>>5)<<3 | (B>>5), split bin = hi5*16 + lo4,
  one-hot(hi5) [px,32] x one-hot(lo4) [px,16] contracted over pixels on the
  PE (PSUM-accumulated bf16 matmuls) -> joint 2-D histogram [32,16] = hist[512].
Phase B (light): per-core sim = xh @ xs^T (xs = zero-padded +-50 frame context),
  diagonal window extract via a stride-164 read over stride-163 rows in a DRAM
  scratch (addr 164*t + l = sim[t, t+l]), PE transpose, FC matmul (W [128,101]).
Host: slices inputs, L2-normalizes histograms between launches, applies
  bias + ReLU (tiny [400,128] tail), reassembles the [4,100,128] output.
"""

import sys

for _p in ("/opt/trn_rl_repo", "/root/.axon_site/_ro/trn_rl_repo"):
    if _p not in sys.path:
        sys.path.append(_p)

import numpy as np

from concourse import bass, bacc, mybir
import concourse.tile as tile
from concourse.bass_utils import run_bass_kernel_spmd
from concourse.masks import make_identity

P = 128
NPIX = 224 * 224        # 50176 pixels per frame plane
FPP = NPIX // P         # 392 pixels per partition
NF = 50                 # frames per core
V1, V2 = 32, 16         # 512 = 32 * 16 bin split
LW = 101
NCORES = 8
F32 = mybir.dt.float32
I32 = mybir.dt.int32
BF16 = mybir.dt.bfloat16
OP = mybir.AluOpType


def build_hist_nc():
    nc = bacc.Bacc("TRN2")
    fr = nc.dram_tensor("fr", [3, NF, NPIX], I32, kind="ExternalInput")
    hist = nc.dram_tensor("hist", [NF, 512], F32, kind="ExternalOutput")
    G = 2                # frames per DVE batch (amortizes per-op overhead)
    FD = G * FPP         # 784 free-dim elements per DVE op

    with tile.TileContext(nc) as tc:
        with (
            tc.tile_pool(name="io", bufs=4) as io,
            tc.tile_pool(name="mid", bufs=2) as mid,
            tc.tile_pool(name="oh", bufs=2) as oh,
            tc.tile_pool(name="cst", bufs=1) as cst,
            tc.tile_pool(name="ps", bufs=2, space="PSUM") as ps,
        ):
            osb = cst.tile([V1, NF * V2], F32)  # [32, 800] result staging

            for t0 in range(0, NF, G):
                r = io.tile([P, FD], I32, tag="ch")
                g = io.tile([P, FD], I32, tag="ch")
                b = io.tile([P, FD], I32, tag="ch")
                for ci, ch in ((0, r), (1, g), (2, b)):
                    nc.sync.dma_start(
                        out=ch[:].rearrange("p (q f) -> p q f", q=G),
                        in_=fr[ci, t0:t0 + G].rearrange("q (p f) -> p q f", p=P))

                # hi5 = (R>>5)*4 + (G>>6) = ((R>>3)&28) | (G>>6)
                # lo4 = ((G>>5)&1)*8 + (B>>5) = ((G>>2)&8) | (B>>5)
                a2 = mid.tile([P, FD], I32, tag="t1")
                nc.vector.tensor_scalar(
                    out=a2[:], in0=r[:], scalar1=3, scalar2=28,
                    op0=OP.logical_shift_right, op1=OP.bitwise_and)
                b2 = mid.tile([P, FD], I32, tag="t2")
                nc.vector.tensor_scalar(
                    out=b2[:], in0=g[:], scalar1=6, scalar2=None,
                    op0=OP.logical_shift_right)
                hi_i = mid.tile([P, FD], I32, tag="t3")
                nc.vector.tensor_tensor(
                    out=hi_i[:], in0=a2[:], in1=b2[:], op=OP.bitwise_or)
                hi_b = mid.tile([P, FD], BF16, tag="tb")
                nc.vector.tensor_copy(out=hi_b[:], in_=hi_i[:])

                c2 = mid.tile([P, FD], I32, tag="t1")
                nc.vector.tensor_scalar(
                    out=c2[:], in0=g[:], scalar1=2, scalar2=8,
                    op0=OP.logical_shift_right, op1=OP.bitwise_and)
                d2 = mid.tile([P, FD], I32, tag="t2")
                nc.vector.tensor_scalar(
                    out=d2[:], in0=b[:], scalar1=5, scalar2=None,
                    op0=OP.logical_shift_right)
                lo_i = mid.tile([P, FD], I32, tag="t3")
                nc.vector.tensor_tensor(
                    out=lo_i[:], in0=c2[:], in1=d2[:], op=OP.bitwise_or)
                lo_b = mid.tile([P, FD], BF16, tag="tb")
                nc.vector.tensor_copy(out=lo_b[:], in_=lo_i[:])

                # one-hot via per-value tensor_scalar is_equal over G frames:
                # bf16 single-src step-1 SBUF -> DVE 4x perf mode.
                A = oh.tile([P, V1 * FD], BF16, tag="A")
                for v in range(V1):
                    nc.vector.tensor_scalar(
                        out=A[:, v * FD:(v + 1) * FD], in0=hi_b[:],
                        scalar1=float(v), scalar2=None, op0=OP.is_equal)
                B = oh.tile([P, V2 * FD], BF16, tag="B")
                for v in range(V2):
                    nc.vector.tensor_scalar(
                        out=B[:, v * FD:(v + 1) * FD], in0=lo_b[:],
                        scalar1=float(v), scalar2=None, op0=OP.is_equal)

                # contract over pixels per frame: hist2d[u, w] += A_qj^T @ B_qj
                Aq = A[:].rearrange("p (v q f) -> p q f v", v=V1, q=G)
                Bq = B[:].rearrange("p (v q f) -> p q f v", v=V2, q=G)
                for q in range(G):
                    hps = ps.tile([V1, V2], F32)
                    for j in range(FPP):
                        nc.tensor.matmul(
                            out=hps[:],
                            lhsT=Aq[:, q, j, :],
                            rhs=Bq[:, q, j, :],
                            start=(j == 0), stop=(j == FPP - 1))
                    t = t0 + q
                    nc.vector.tensor_copy(
                        out=osb[:, t * V2:(t + 1) * V2], in_=hps[:])

            nc.sync.dma_start(
                out=hist[:].rearrange("t (u w) -> u t w", u=V1),
                in_=osb[:].rearrange("u (t w) -> u t w", w=V2))
    nc.compile()
    return nc


def build_fc_nc():
    """sim2 = xh @ xs^T [50,150]; win[t,l] = sim2[t, t+l]; out = relu(win@W^T + b)."""
    nc = bacc.Bacc("TRN2")
    # columns 0:50 = x_half^T, 50:200 = padded-context^T (one DMA -> one sem wait)
    xallT = nc.dram_tensor("xallT", [512, 200], F32, kind="ExternalInput")
    wT = nc.dram_tensor("wT", [LW, P], F32, kind="ExternalInput")
    out = nc.dram_tensor("out", [NF, P], F32, kind="ExternalOutput")
    # rows written at stride 163 (sim2[t] at 163*t), diagonal read back at
    # stride 164: addr 164*t + l = 163*t + (t+l) = sim2[t, t+l]  (no overlap)
    scratch = nc.dram_tensor("scratch", [NF * 164], F32, kind="Internal")

    with tile.TileContext(nc) as tc:
        with (
            tc.tile_pool(name="sb", bufs=1) as sb,
            tc.tile_pool(name="ps", bufs=1, space="PSUM") as ps,
        ):
            xa_sb = sb.tile([P, 4 * 200], F32)
            nc.sync.dma_start(
                out=xa_sb[:].rearrange("p (a t) -> p a t", a=4),
                in_=xallT[:].rearrange("(a p) t -> p a t", p=P))
            wt_sb = sb.tile([LW, P], F32)
            nc.sync.dma_start(out=wt_sb[:], in_=wT[:])

            sim_ps = ps.tile([NF, 150], F32)
            for a in range(4):
                nc.tensor.matmul(
                    out=sim_ps[:],
                    lhsT=xa_sb[:, a * 200:a * 200 + NF],
                    rhs=xa_sb[:, a * 200 + NF:(a + 1) * 200],
                    start=(a == 0), stop=(a == 3))
            sim_sb = sb.tile([NF, 150], F32)
            nc.vector.tensor_copy(out=sim_sb[:], in_=sim_ps[:])

            # row t of sim2 lands at flat offset 163*t
            nc.gpsimd.dma_start(
                out=scratch[0:NF * 163].rearrange("(t c) -> t c", c=163)[:, 0:150],
                in_=sim_sb[:])
            # diagonal: win[t, l] = scratch[164*t + l] = sim2[t, t+l]
            win_sb = sb.tile([NF, LW], F32)
            nc.gpsimd.dma_start(
                out=win_sb[:],
                in_=scratch[0:NF * 164].rearrange("(t c) -> t c", c=164)[:, 0:LW])

            # transpose win [50, 101] -> [101, 50] on the PE
            ident = sb.tile([NF, NF], F32)
            make_identity(nc, ident[:])
            win_ps = ps.tile([LW, NF], F32)
            nc.tensor.transpose(out=win_ps[:], in_=win_sb[:], identity=ident[:])
            win2 = sb.tile([LW, NF], F32)
            nc.vector.tensor_copy(out=win2[:], in_=win_ps[:])
            wt2 = sb.tile([LW, P], F32)
            nc.vector.tensor_copy(out=wt2[:], in_=wt_sb[:])

            fc_ps = ps.tile([P, NF], F32)
            nc.tensor.matmul(out=fc_ps[:], lhsT=wt2[:], rhs=win2[:],
                             start=True, stop=True)
            res = sb.tile([P, NF], F32)
            nc.vector.tensor_copy(out=res[:], in_=fc_ps[:])
            # bias + relu applied on host (tiny); avoids a 2-wait Activation
            nc.sync.dma_start(out=out[:].rearrange("t o -> o t"), in_=res[:])
    nc.compile()
    return nc


_NC_CACHE = {}


def _get_nc(key, builder):
    if key not in _NC_CACHE:
        _NC_CACHE[key] = builder()
    return _NC_CACHE[key]


def kernel(frames, W, b):
    frames = np.asarray(frames, dtype=np.int32)
    W = np.asarray(W, dtype=np.float32)
    b = np.asarray(b, dtype=np.float32)
    Bn, _, T = frames.shape[:3]  # [4, 3, 100, 224, 224]

    nc_a = _get_nc("A", build_hist_nc)
    in_maps = []
    for c in range(NCORES):
        bi, h = c // 2, c % 2
        sl = frames[bi, :, h * NF:(h + 1) * NF].reshape(3, NF, NPIX)
        in_maps.append({"fr": np.ascontiguousarray(sl)})
    res_a = run_bass_kernel_spmd(nc_a, in_maps, list(range(NCORES))).results

    counts = np.zeros((Bn, T, 512), np.float32)
    for c in range(NCORES):
        bi, h = c // 2, c % 2
        counts[bi, h * NF:(h + 1) * NF] = res_a[c]["hist"]
    xn = counts / np.linalg.norm(counts, axis=2, keepdims=True)

    nc_b = _get_nc("B", build_fc_nc)
    wT = np.ascontiguousarray(W.T)           # [101, 128]
    in_maps = []
    for c in range(NCORES):
        bi, h = c // 2, c % 2
        t0 = h * NF
        xall = np.zeros((200, 512), np.float32)
        xall[0:NF] = xn[bi, t0:t0 + NF]                  # x_half
        xall[NF + 50 - t0:NF + 50 - t0 + T] = xn[bi]     # xs[s'] = xn[s'+t0-50]
        in_maps.append({"xallT": np.ascontiguousarray(xall.T), "wT": wT})
    res_b = run_bass_kernel_spmd(nc_b, in_maps, list(range(NCORES))).results

    outp = np.zeros((Bn, T, P), np.float32)
    for c in range(NCORES):
        bi, h = c // 2, c % 2
        outp[bi, h * NF:(h + 1) * NF] = res_b[c]["out"]
    outp = np.maximum(outp + b[None, None, :], 0.0)
    return outp
